# revision 4
# baseline (speedup 1.0000x reference)
"""Trainium2 Bass kernel for nn_Custom_RNN (LSTM-ish cell + vocab logits).

Computation (faithful to the reference, including its quirks):
    xe = emb[x]
    pre  = xe@Wxi.T + (h@Whi.T + bhi) + (c@Whc.T + bhc)
    i = f = tanh(pre)
    c_new = i * (c + tanh(xe@Wxc.T + h@Whc.T + bhc))
    o = tanh(xe@Wxo.T + (h@Who.T + bho) + (c_new@Who.T + bho))
    h_new = o * tanh(c_new)
    logits = h_new@Wy.T + by
    returns (logits, h_new, c_new)

Sharding: data-parallel over batch (256 rows/core) for the recurrent part,
then AllGather of h_new^T (bf16) and vocab-parallel logits GEMM
(6400 padded vocab rows per core).  All GEMMs run in bf16 with fp32 PSUM
accumulation; elementwise math in fp32.  Weights are cast to bf16 on the
host and transposed on-device via DMA-xbar transpose (contraction dim must
sit on SBUF partitions for the PE).
"""

import sys

sys.path.insert(0, "/opt/trn_rl_repo")

import numpy as np
import ml_dtypes

V, E, H, B = 50257, 1024, 1024, 2048
NCORES = 8
BS = B // NCORES  # 256 batch rows per core
MT = BS // 128  # 2 m-tiles per core
KT = H // 128  # 8 k-tiles over the hidden/embedding dim
VP = 6400  # padded vocab rows per core (8*6400 = 51200 >= 50257)
VPAD = VP * NCORES
# logits N-chunks per core: 12 x 512 + 1 x 256
CHUNKS = [(i * 512, 512) for i in range(12)] + [(6144, 256)]

BF16 = ml_dtypes.bfloat16

_STATE = {}


def _build_program():
    import concourse.bass as bass
    import concourse.mybir as mybir
    import concourse.tile as tile
    from concourse import bacc
    from contextlib import ExitStack

    dt = mybir.dt
    nc = bacc.Bacc("TRN2", target_bir_lowering=False, debug=False,
                   num_devices=NCORES)

    # ---- DRAM I/O ----
    xe_d = nc.dram_tensor("xe_b", [BS, E], dt.bfloat16, kind="ExternalInput")
    h_d = nc.dram_tensor("h_b", [BS, H], dt.bfloat16, kind="ExternalInput")
    cb_d = nc.dram_tensor("c_b", [BS, H], dt.bfloat16, kind="ExternalInput")
    cf_d = nc.dram_tensor("c_f", [BS, H], dt.float32, kind="ExternalInput")
    gw_d = {
        name: nc.dram_tensor(name, [H, E], dt.bfloat16, kind="ExternalInput")
        for name in ("Wxi", "Whi", "Wxc", "Whc", "Wxo", "Who")
    }
    bias_d = {
        name: nc.dram_tensor(name, [1, H], dt.bfloat16, kind="ExternalInput")
        for name in ("bhi", "bhc", "bho")
    }
    wy_d = nc.dram_tensor("Wy_b", [VP, H], dt.bfloat16, kind="ExternalInput")
    by_d = nc.dram_tensor("by_b", [1, VP], dt.bfloat16, kind="ExternalInput")

    logits_d = nc.dram_tensor("logits", [B, VP], dt.float32,
                              kind="ExternalOutput")
    hnew_d = nc.dram_tensor("h_new", [BS, H], dt.float32,
                            kind="ExternalOutput")
    cnew_d = nc.dram_tensor("c_new", [BS, H], dt.float32,
                            kind="ExternalOutput")

    f32 = dt.float32
    bf = dt.bfloat16

    with tile.TileContext(nc) as tc:
        with ExitStack() as top:
            dram = top.enter_context(tc.tile_pool(name="dram", bufs=1,
                                                  space="DRAM"))
            const = top.enter_context(tc.tile_pool(name="const", bufs=1))

            ones = const.tile([1, 128], bf)
            nc.vector.memset(ones[:], 1.0)
            brow = {}
            for name in ("bhi", "bhc", "bho"):
                t = const.tile([1, H], bf, name=f"{name}_row")
                nc.sync.dma_start(t[:], bias_d[name].ap())
                brow[name] = t
            by_row = const.tile([1, VP], bf)
            nc.sync.dma_start(by_row[:], by_d.ap())

            cc_in = dram.tile([KT * 128, BS], bf)  # h_new^T of this core
            cc_out = dram.tile([NCORES * KT * 128, BS], bf,
                               addr_space="Shared")

            # ================= Phase A: recurrent cell (batch-sharded) ====
            with ExitStack() as sa:
                gwp = sa.enter_context(tc.tile_pool(name="gw", bufs=1))
                act = sa.enter_context(tc.tile_pool(name="act", bufs=1))
                # 2 tags x 4 bufs x [128,512]f32 = exactly the 8 PSUM banks
                gps = sa.enter_context(tc.tile_pool(name="gpsum", bufs=4,
                                                    space="PSUM"))

                # gate weights, transposed to [e(128), k, o] via DMA xbar
                gwT = {}
                for name in ("Wxi", "Whi", "Whc", "Wxc", "Wxo", "Who"):
                    t = gwp.tile([128, KT, H], bf, name=f"{name}T")
                    for k in range(KT):
                        nc.sync.dma_start(
                            t[:, k, :],
                            gw_d[name].ap()[:, k * 128:(k + 1) * 128],
                            transpose=True,
                        )
                    gwT[name] = t

                # activations, transposed to [e(128), k, m] via DMA xbar
                aT = {}
                for name, d in (("xe", xe_d), ("h", h_d), ("c", cb_d)):
                    t = act.tile([128, KT, BS], bf, name=f"{name}T")
                    for k in range(KT):
                        nc.sync.dma_start(
                            t[:, k, :],
                            d.ap()[:, k * 128:(k + 1) * 128],
                            transpose=True,
                        )
                    aT[name] = t

                cf = act.tile([128, MT, H], f32)
                nc.sync.dma_start(
                    cf[:], cf_d.ap().rearrange("(mt p) e -> p mt e", p=128))

                i_g = act.tile([128, MT, H], f32)    # i == f gate
                tcg = act.tile([128, MT, H], f32)    # tanh(inner_c)
                cn = act.tile([128, MT, H], f32)     # c_new
                tmp = act.tile([128, MT, H], f32)
                og = act.tile([128, MT, H], f32)     # o gate
                hn = act.tile([128, MT, H], f32)     # h_new
                cnb = act.tile([128, MT, H], bf)
                hnb = act.tile([128, MT, H], bf)
                cnT = act.tile([128, KT, BS], bf)
                hnT = act.tile([128, KT, BS], bf)

                NCH = H // 512  # 2 n-chunks of 512 over the gate outputs

                def gate_mms(ps, mt, nch, terms, biases, hold=False):
                    """accumulate sum_k lhsT.T@rhs (+ broadcast biases)."""
                    first = True
                    for bname in biases:
                        nc.tensor.matmul(
                            ps[:],
                            ones[:],
                            brow[bname][:, nch * 512:(nch + 1) * 512],
                            start=first, stop=False)
                        first = False
                    n_mms = len(terms) * KT
                    done = 0
                    for a_t, w_t in terms:
                        for k in range(KT):
                            done += 1
                            nc.tensor.matmul(
                                ps[:],
                                a_t[:, k, mt * 128:(mt + 1) * 128],
                                w_t[:, k, nch * 512:(nch + 1) * 512],
                                start=first,
                                stop=(not hold) and done == n_mms)
                            first = False

                # pre-gate -> i (=f)
                for mt in range(MT):
                    for nchu in range(NCH):
                        ps = gps.tile([128, 512], f32, tag="ps")
                        gate_mms(ps, mt, nchu,
                                 [(aT["xe"], gwT["Wxi"]),
                                  (aT["h"], gwT["Whi"]),
                                  (aT["c"], gwT["Whc"])],
                                 ["bhi", "bhc"])
                        nc.scalar.activation(
                            i_g[:, mt, nchu * 512:(nchu + 1) * 512], ps[:],
                            mybir.ActivationFunctionType.Tanh)

                # inner_c -> tanh
                for mt in range(MT):
                    for nchu in range(NCH):
                        ps = gps.tile([128, 512], f32, tag="ps")
                        gate_mms(ps, mt, nchu,
                                 [(aT["xe"], gwT["Wxc"]),
                                  (aT["h"], gwT["Whc"])],
                                 ["bhc"])
                        nc.scalar.activation(
                            tcg[:, mt, nchu * 512:(nchu + 1) * 512], ps[:],
                            mybir.ActivationFunctionType.Tanh)

                # o partial: xe@Wxo + h@Who + 2*bho (held in PSUM)
                ops_tiles = {}
                for mt in range(MT):
                    for nchu in range(NCH):
                        ps = gps.tile([128, 512], f32, tag="ops")
                        gate_mms(ps, mt, nchu,
                                 [(aT["xe"], gwT["Wxo"]),
                                  (aT["h"], gwT["Who"])],
                                 ["bho", "bho"], hold=True)
                        ops_tiles[(mt, nchu)] = ps

                # c_new = i * (c + tanh(inner))
                nc.vector.tensor_tensor(tmp[:], cf[:], tcg[:],
                                        mybir.AluOpType.add)
                nc.vector.tensor_tensor(cn[:], i_g[:], tmp[:],
                                        mybir.AluOpType.mult)
                nc.sync.dma_start(
                    cnew_d.ap().rearrange("(mt p) e -> p mt e", p=128), cn[:])
                nc.vector.tensor_copy(cnb[:], cn[:])
                for mt in range(MT):
                    for k in range(KT):
                        nc.sync.dma_start(
                            cnT[:, k, mt * 128:(mt + 1) * 128],
                            cnb[:, mt, k * 128:(k + 1) * 128],
                            transpose=True)

                # finish o gate: += c_new@Who
                for mt in range(MT):
                    for nchu in range(NCH):
                        ps = ops_tiles[(mt, nchu)]
                        for k in range(KT):
                            nc.tensor.matmul(
                                ps[:],
                                cnT[:, k, mt * 128:(mt + 1) * 128],
                                gwT["Who"][:, k, nchu * 512:(nchu + 1) * 512],
                                start=False, stop=(k == KT - 1))
                        nc.scalar.activation(
                            og[:, mt, nchu * 512:(nchu + 1) * 512], ps[:],
                            mybir.ActivationFunctionType.Tanh)

                # h_new = o * tanh(c_new)
                nc.scalar.activation(tmp[:], cn[:],
                                     mybir.ActivationFunctionType.Tanh)
                nc.vector.tensor_tensor(hn[:], og[:], tmp[:],
                                        mybir.AluOpType.mult)
                nc.sync.dma_start(
                    hnew_d.ap().rearrange("(mt p) e -> p mt e", p=128), hn[:])
                nc.vector.tensor_copy(hnb[:], hn[:])
                for mt in range(MT):
                    for k in range(KT):
                        nc.sync.dma_start(
                            hnT[:, k, mt * 128:(mt + 1) * 128],
                            hnb[:, mt, k * 128:(k + 1) * 128],
                            transpose=True)
                nc.sync.dma_start(
                    cc_in.rearrange("(k p) m -> p k m", p=128), hnT[:])

            # ================= Phase B: AllGather h_new^T =================
            nc.gpsimd.collective_compute(
                "AllGather",
                mybir.AluOpType.bypass,
                replica_groups=[list(range(NCORES))],
                ins=[cc_in.opt()],
                outs=[cc_out.opt()],
            )

            # ================= Phase C: logits (vocab-sharded) ============
            with ExitStack() as sb:
                htp = sb.enter_context(tc.tile_pool(name="ht", bufs=1))
                wyp = sb.enter_context(tc.tile_pool(name="wy", bufs=3))
                bfp = sb.enter_context(tc.tile_pool(name="bfull", bufs=1))
                stg = sb.enter_context(tc.tile_pool(name="stage", bufs=6))
                lps = sb.enter_context(tc.tile_pool(name="lpsum", bufs=6,
                                                    space="PSUM"))

                # full h_new^T: [e(128), k, core, m]
                hT = htp.tile([128, KT, NCORES, BS], bf)
                ccv = cc_out.rearrange("(r k p) m -> p r k m", r=NCORES,
                                       k=KT)
                for r in range(NCORES):
                    nc.sync.dma_start(hT[:, :, r, :], ccv[:, r])

                # bias row -> broadcast to 128 partitions via K=1 matmul
                bias_full = bfp.tile([128, VP], f32)
                for c0, cs in CHUNKS:
                    psb = lps.tile([128, 512], f32, tag="psb", bufs=2)
                    nc.tensor.matmul(psb[:, :cs], ones[:],
                                     by_row[:, c0:c0 + cs],
                                     start=True, stop=True)
                    nc.vector.tensor_copy(bias_full[:, c0:c0 + cs],
                                          psb[:, :cs])

                for c0, cs in CHUNKS:
                    wyT = wyp.tile([128, KT, 512], bf, tag="wyT")
                    for k in range(KT):
                        nc.sync.dma_start(
                            wyT[:, k, :cs],
                            wy_d.ap()[c0:c0 + cs, k * 128:(k + 1) * 128],
                            transpose=True)
                    for mt in range(B // 128):
                        r, ml = divmod(mt, MT)
                        ps = lps.tile([128, 512], f32, tag="ps")
                        for k in range(KT):
                            nc.tensor.matmul(
                                ps[:, :cs],
                                hT[:, k, r, ml * 128:(ml + 1) * 128],
                                wyT[:, k, :cs],
                                start=(k == 0), stop=(k == KT - 1))
                        out_t = stg.tile([128, 512], f32, tag="out")
                        nc.vector.tensor_tensor(out_t[:, :cs], ps[:, :cs],
                                                bias_full[:, c0:c0 + cs],
                                                mybir.AluOpType.add)
                        nc.sync.dma_start(
                            logits_d.ap()[mt * 128:(mt + 1) * 128,
                                          c0:c0 + cs],
                            out_t[:, :cs])

    nc.compile()
    return nc


def _get_runner():
    """Build (once) a persistent jitted SPMD runner over the 8 cores."""
    if "runner" in _STATE:
        return _STATE["runner"]

    import jax
    import jax.numpy as jnp
    import concourse.mybir as mybir
    from jax.sharding import Mesh, PartitionSpec, NamedSharding
    from jax.experimental.shard_map import shard_map
    from concourse import bass2jax

    nc = _build_program()
    bass2jax.install_neuronx_cc_hook()

    partition_name = (nc.partition_id_tensor.name
                      if nc.partition_id_tensor else None)
    in_names, out_names, out_avals = [], [], []
    for alloc in nc.m.functions[0].allocations:
        if not isinstance(alloc, mybir.MemoryLocationSet):
            continue
        name = alloc.memorylocations[0].name
        if alloc.kind == "ExternalInput":
            if name != partition_name:
                in_names.append(name)
        elif alloc.kind == "ExternalOutput":
            out_names.append(name)
            out_avals.append(jax.core.ShapedArray(
                tuple(alloc.tensor_shape), mybir.dt.np(alloc.dtype)))

    n_params = len(in_names)
    n_outs = len(out_avals)
    all_in_names = list(in_names) + list(out_names)
    if partition_name is not None:
        all_in_names.append(partition_name)

    # which inputs are identical on all cores (replicated)?
    REPLICATED = {"Wxi", "Whi", "Wxc", "Whc", "Wxo", "Who",
                  "bhi", "bhc", "bho"}

    def _body(*args):
        operands = list(args)
        if partition_name is not None:
            operands.append(bass2jax.partition_id_tensor())
        outs = bass2jax._bass_exec_p.bind(
            *operands,
            out_avals=tuple(out_avals),
            in_names=tuple(all_in_names),
            out_names=tuple(out_names),
            lowering_input_output_aliases=(),
            sim_require_finite=True,
            sim_require_nnan=True,
            nc=nc,
        )
        return tuple(outs)

    devices = jax.devices()[:NCORES]
    mesh = Mesh(np.asarray(devices), ("core",))
    shard = NamedSharding(mesh, PartitionSpec("core"))
    repl = NamedSharding(mesh, PartitionSpec())

    in_specs = tuple(
        PartitionSpec() if n in REPLICATED else PartitionSpec("core")
        for n in in_names) + (PartitionSpec("core"),) * n_outs
    out_specs = (PartitionSpec("core"),) * n_outs
    donate = tuple(range(n_params, n_params + n_outs))

    sharded = jax.jit(
        shard_map(_body, mesh=mesh, in_specs=in_specs, out_specs=out_specs,
                  check_rep=False),
        donate_argnums=donate, keep_unused=True)

    zero_fn = jax.jit(
        lambda: tuple(
            jnp.zeros((NCORES * a.shape[0], *a.shape[1:]), a.dtype)
            for a in out_avals),
        out_shardings=tuple(shard for _ in out_avals))

    def put(name, arr):
        return jax.device_put(
            np.asarray(arr), repl if name in REPLICATED else shard)

    runner = {
        "in_names": in_names,
        "out_names": out_names,
        "sharded": sharded,
        "zero_fn": zero_fn,
        "put": put,
    }
    _STATE["runner"] = runner
    return runner


def _prep_inputs(x, h, c, emb, Wxi, Whi, bhi, Wxc, Whc, bhc, Wxo, Who, bho,
                 Wy, by):
    """Host-side shard/layout prep. Returns {name: global array}."""
    x = np.asarray(x)
    emb = np.asarray(emb, dtype=np.float32)
    xe = emb[x]  # [B, E] gather
    h = np.asarray(h, dtype=np.float32)
    c = np.asarray(c, dtype=np.float32)

    wy_pad = np.zeros((VPAD, H), dtype=BF16)
    wy_pad[:V] = np.asarray(Wy, dtype=np.float32).astype(BF16)
    by_pad = np.zeros((NCORES, 1, VP), dtype=BF16)
    by_pad.reshape(-1)[:V] = np.asarray(by, dtype=np.float32).astype(BF16)
    # note: reshape(-1)[:V] fills row-major across cores: core r cols
    # [r*VP, (r+1)*VP) -> matches wy_pad row slices. (by is 1-D length V)

    feed = {
        "xe_b": xe.astype(BF16),
        "h_b": h.astype(BF16),
        "c_b": c.astype(BF16),
        "c_f": c,
        "Wy_b": wy_pad,
        "by_b": by_pad.reshape(NCORES * 1, VP),
        "Wxi": np.asarray(Wxi, np.float32).astype(BF16),
        "Whi": np.asarray(Whi, np.float32).astype(BF16),
        "Wxc": np.asarray(Wxc, np.float32).astype(BF16),
        "Whc": np.asarray(Whc, np.float32).astype(BF16),
        "Wxo": np.asarray(Wxo, np.float32).astype(BF16),
        "Who": np.asarray(Who, np.float32).astype(BF16),
        "bhi": np.asarray(bhi, np.float32).astype(BF16).reshape(1, H),
        "bhc": np.asarray(bhc, np.float32).astype(BF16).reshape(1, H),
        "bho": np.asarray(bho, np.float32).astype(BF16).reshape(1, H),
    }
    return feed


def _run_device(feed_dev):
    r = _get_runner()
    zeros = r["zero_fn"]()
    args = [feed_dev[n] for n in r["in_names"]] + list(zeros)
    outs = r["sharded"](*args)
    return outs


def kernel(x, h, c, emb, Wxi, Whi, bhi, Wxc, Whc, bhc, Wxo, Who, bho, Wy, by):
    r = _get_runner()
    feed = _prep_inputs(x, h, c, emb, Wxi, Whi, bhi, Wxc, Whc, bhc,
                        Wxo, Who, bho, Wy, by)
    feed_dev = {n: r["put"](n, feed[n]) for n in r["in_names"]}
    outs = _run_device(feed_dev)
    by_name = dict(zip(r["out_names"], outs))

    logits_g = np.asarray(by_name["logits"])  # [8*2048, 6400]
    logits = np.concatenate(
        [logits_g[i * B:(i + 1) * B] for i in range(NCORES)],
        axis=1)[:, :V]
    h_new = np.asarray(by_name["h_new"]).reshape(B, H)
    c_new = np.asarray(by_name["c_new"]).reshape(B, H)
    return logits.astype(np.float32), h_new, c_new


# revision 12
# speedup vs baseline: 7311.4001x; 7311.4001x over previous
"""Trainium2 Bass kernel for nn_Custom_RNN (LSTM-ish cell + vocab logits).

Computation (faithful to the reference, including its quirks):
    xe = emb[x]
    pre  = xe@Wxi.T + (h@Whi.T + bhi) + (c@Whc.T + bhc)
    i = f = tanh(pre)
    c_new = i * (c + tanh(xe@Wxc.T + h@Whc.T + bhc))
    o = tanh(xe@Wxo.T + (h@Who.T + bho) + (c_new@Who.T + bho))
    h_new = o * tanh(c_new)
    logits = h_new@Wy.T + by
    returns (logits, h_new, c_new)

Sharding: data-parallel over batch (256 rows/core) for the recurrent part,
then AllGather of h_new^T (bf16) and vocab-parallel logits GEMM
(6400 padded vocab rows per core).  All GEMMs run in bf16 with fp32 PSUM
accumulation; elementwise math in fp32.  Weights/activations are cast to
bf16 and pre-transposed on the host where cheap; the big Wy weight is
transposed on-device via DMA-xbar transpose (the PE needs the contraction
dim on SBUF partitions).
"""

import sys

sys.path.insert(0, "/opt/trn_rl_repo")

import numpy as np
import ml_dtypes

V, E, H, B = 50257, 1024, 1024, 2048
NCORES = 8
BS = B // NCORES  # 256 batch rows per core
MT = BS // 128  # 2 m-tiles per core
KT = H // 128  # 8 k-tiles over the hidden/embedding dim
VP = 6400  # padded vocab rows per core (8*6400 = 51200 >= 50257)
VPAD = VP * NCORES
# Wy transpose groups (device DMA-xbar): 3 x 2048 + 1 x 256
GROUPS = [(0, 2048), (2048, 2048), (4096, 2048), (6144, 256)]
# logits N-chunks (PSUM bank width): 12 x 512 + 1 x 256
CHUNKS = [(i * 512, 512) for i in range(12)] + [(6144, 256)]

BF16 = ml_dtypes.bfloat16

_STATE = {}


def _build_program(use_collective=True):
    import concourse.bass as bass
    import concourse.mybir as mybir
    import concourse.tile as tile
    from concourse import bacc
    from contextlib import ExitStack

    dt = mybir.dt
    nc = bacc.Bacc("TRN2", target_bir_lowering=False, debug=False,
                   num_devices=NCORES)

    # ---- DRAM I/O (all *T tensors arrive pre-transposed from the host) ----
    xeT_d = nc.dram_tensor("xeT_b", [H, BS], dt.bfloat16, kind="ExternalInput")
    hT_d = nc.dram_tensor("hT_b", [H, BS], dt.bfloat16, kind="ExternalInput")
    cT_d = nc.dram_tensor("cT_b", [H, BS], dt.bfloat16, kind="ExternalInput")
    cf_d = nc.dram_tensor("c_f", [BS, H], dt.float32, kind="ExternalInput")
    gw_d = {
        name: nc.dram_tensor(name + "T", [E, H], dt.bfloat16,
                             kind="ExternalInput")
        for name in ("Wxi", "Whi", "Wxc", "Whc", "Wxo", "Who")
    }
    bias_d = {
        name: nc.dram_tensor(name, [1, H], dt.bfloat16, kind="ExternalInput")
        for name in ("bhi", "bhc", "bho")
    }
    wy_d = nc.dram_tensor("Wy_b", [VP, H], dt.bfloat16, kind="ExternalInput")
    by_d = nc.dram_tensor("by_b", [1, VP], dt.bfloat16, kind="ExternalInput")

    logits_d = nc.dram_tensor("logits", [B, VP], dt.float32,
                              kind="ExternalOutput")
    hnew_d = nc.dram_tensor("h_new", [BS, H], dt.float32,
                            kind="ExternalOutput")
    cnew_d = nc.dram_tensor("c_new", [BS, H], dt.float32,
                            kind="ExternalOutput")

    f32 = dt.float32
    bf = dt.bfloat16

    with tile.TileContext(nc) as tc:
        with ExitStack() as top:
            dram = top.enter_context(tc.tile_pool(name="dram", bufs=1,
                                                  space="DRAM"))
            const = top.enter_context(tc.tile_pool(name="const", bufs=1))

            ones = const.tile([1, 128], bf)
            nc.vector.memset(ones[:], 1.0)
            brow = {}
            for name in ("bhi", "bhc", "bho"):
                t = const.tile([1, H], bf, name=f"{name}_row")
                nc.scalar.dma_start(t[:], bias_d[name].ap())
                brow[name] = t
            by_row = const.tile([1, VP], bf)
            nc.scalar.dma_start(by_row[:], by_d.ap())

            cc_in = dram.tile([KT * 128, BS], bf)  # h_new^T of this core
            cc_out = dram.tile(
                [NCORES * KT * 128, BS], bf,
                addr_space="Shared" if use_collective else "Local")

            # ================= Phase A: recurrent cell (batch-sharded) ====
            with ExitStack() as sa:
                gwp = sa.enter_context(tc.tile_pool(name="gw", bufs=1))
                act = sa.enter_context(tc.tile_pool(name="act", bufs=1))
                # 2 tags x 4 bufs x [128,512]f32 = exactly the 8 PSUM banks
                gps = sa.enter_context(tc.tile_pool(name="gpsum", bufs=4,
                                                    space="PSUM"))

                # gate weights (already [e, o] in DRAM) -> [e(128), k, o]
                gwT = {}
                for name in ("Wxi", "Whi", "Whc", "Wxc", "Wxo", "Who"):
                    t = gwp.tile([128, KT, H], bf, name=f"{name}T")
                    nc.sync.dma_start(
                        t[:], gw_d[name].ap().rearrange("(k p) o -> p k o",
                                                        p=128))
                    gwT[name] = t

                # activations (already [e, m] in DRAM) -> [e(128), k, m]
                aT = {}
                for name, d in (("xe", xeT_d), ("h", hT_d), ("c", cT_d)):
                    t = act.tile([128, KT, BS], bf, name=f"{name}T")
                    nc.sync.dma_start(
                        t[:], d.ap().rearrange("(k p) m -> p k m", p=128))
                    aT[name] = t

                cf = act.tile([128, MT, H], f32)
                nc.sync.dma_start(
                    cf[:], cf_d.ap().rearrange("(mt p) e -> p mt e", p=128))

                i_g = act.tile([128, MT, H], f32)    # i == f gate
                tcg = act.tile([128, MT, H], f32)    # tanh(inner_c)
                cn = act.tile([128, MT, H], f32)     # c_new
                tmp = act.tile([128, MT, H], f32)
                og = act.tile([128, MT, H], f32)     # o gate
                hn = act.tile([128, MT, H], f32)     # h_new
                cnb = act.tile([128, MT, H], bf)
                hnb = act.tile([128, MT, H], bf)
                cnT = act.tile([128, KT, BS], bf)
                hnT = act.tile([128, KT, BS], bf)

                NCH = H // 512  # 2 n-chunks of 512 over the gate outputs

                def gate_mms(ps, mt, nch, terms, biases, hold=False):
                    """accumulate sum_k lhsT.T@rhs (+ broadcast biases)."""
                    first = True
                    for bname in biases:
                        nc.tensor.matmul(
                            ps[:],
                            ones[:],
                            brow[bname][:, nch * 512:(nch + 1) * 512],
                            start=first, stop=False)
                        first = False
                    n_mms = len(terms) * KT
                    done = 0
                    for a_t, w_t in terms:
                        for k in range(KT):
                            done += 1
                            nc.tensor.matmul(
                                ps[:],
                                a_t[:, k, mt * 128:(mt + 1) * 128],
                                w_t[:, k, nch * 512:(nch + 1) * 512],
                                start=first,
                                stop=(not hold) and done == n_mms)
                            first = False

                # pre-gate -> i (=f)
                for mt in range(MT):
                    for nchu in range(NCH):
                        ps = gps.tile([128, 512], f32, tag="ps")
                        gate_mms(ps, mt, nchu,
                                 [(aT["xe"], gwT["Wxi"]),
                                  (aT["h"], gwT["Whi"]),
                                  (aT["c"], gwT["Whc"])],
                                 ["bhi", "bhc"])
                        nc.scalar.activation(
                            i_g[:, mt, nchu * 512:(nchu + 1) * 512], ps[:],
                            mybir.ActivationFunctionType.Tanh)

                # inner_c -> tanh
                for mt in range(MT):
                    for nchu in range(NCH):
                        ps = gps.tile([128, 512], f32, tag="ps")
                        gate_mms(ps, mt, nchu,
                                 [(aT["xe"], gwT["Wxc"]),
                                  (aT["h"], gwT["Whc"])],
                                 ["bhc"])
                        nc.scalar.activation(
                            tcg[:, mt, nchu * 512:(nchu + 1) * 512], ps[:],
                            mybir.ActivationFunctionType.Tanh)

                # o partial: xe@Wxo + h@Who + 2*bho (held in PSUM)
                ops_tiles = {}
                for mt in range(MT):
                    for nchu in range(NCH):
                        ps = gps.tile([128, 512], f32, tag="ops")
                        gate_mms(ps, mt, nchu,
                                 [(aT["xe"], gwT["Wxo"]),
                                  (aT["h"], gwT["Who"])],
                                 ["bho", "bho"], hold=True)
                        ops_tiles[(mt, nchu)] = ps

                # c_new = i * (c + tanh(inner))
                nc.vector.tensor_tensor(tmp[:], cf[:], tcg[:],
                                        mybir.AluOpType.add)
                nc.vector.tensor_tensor(cn[:], i_g[:], tmp[:],
                                        mybir.AluOpType.mult)
                nc.sync.dma_start(
                    cnew_d.ap().rearrange("(mt p) e -> p mt e", p=128), cn[:])
                nc.vector.tensor_copy(cnb[:], cn[:])
                for mt in range(MT):
                    for k in range(KT):
                        nc.scalar.dma_start(
                            cnT[:, k, mt * 128:(mt + 1) * 128],
                            cnb[:, mt, k * 128:(k + 1) * 128],
                            transpose=True)

                # finish o gate: += c_new@Who
                for mt in range(MT):
                    for nchu in range(NCH):
                        ps = ops_tiles[(mt, nchu)]
                        for k in range(KT):
                            nc.tensor.matmul(
                                ps[:],
                                cnT[:, k, mt * 128:(mt + 1) * 128],
                                gwT["Who"][:, k, nchu * 512:(nchu + 1) * 512],
                                start=False, stop=(k == KT - 1))
                        nc.scalar.activation(
                            og[:, mt, nchu * 512:(nchu + 1) * 512], ps[:],
                            mybir.ActivationFunctionType.Tanh)

                # h_new = o * tanh(c_new)
                nc.scalar.activation(tmp[:], cn[:],
                                     mybir.ActivationFunctionType.Tanh)
                nc.vector.tensor_tensor(hn[:], og[:], tmp[:],
                                        mybir.AluOpType.mult)
                nc.sync.dma_start(
                    hnew_d.ap().rearrange("(mt p) e -> p mt e", p=128), hn[:])
                nc.vector.tensor_copy(hnb[:], hn[:])
                for mt in range(MT):
                    for k in range(KT):
                        nc.scalar.dma_start(
                            hnT[:, k, mt * 128:(mt + 1) * 128],
                            hnb[:, mt, k * 128:(k + 1) * 128],
                            transpose=True)
                nc.sync.dma_start(
                    cc_in.rearrange("(k p) m -> p k m", p=128), hnT[:])

            # ================= Phase B: AllGather h_new^T =================
            if use_collective:
                nc.gpsimd.collective_compute(
                    "AllGather",
                    mybir.AluOpType.bypass,
                    replica_groups=[list(range(NCORES))],
                    ins=[cc_in.opt()],
                    outs=[cc_out.opt()],
                )
            else:
                # structural stand-in for local timeline analysis only
                ccov = cc_out.rearrange("(r q) m -> r q m", r=NCORES)
                for rr in range(NCORES):
                    nc.sync.dma_start(ccov[rr], cc_in[:])

            # ================= Phase C: logits (vocab-sharded) ============
            with ExitStack() as sb:
                htp = sb.enter_context(tc.tile_pool(name="ht", bufs=1))
                wyp = sb.enter_context(tc.tile_pool(name="wy", bufs=2))
                bfp = sb.enter_context(tc.tile_pool(name="bfull", bufs=1))
                stg = sb.enter_context(tc.tile_pool(name="stage", bufs=2))
                lps = sb.enter_context(tc.tile_pool(name="lpsum", bufs=6,
                                                    space="PSUM"))

                # full h_new^T: [e(128), k, core, m]
                hT = htp.tile([128, KT, NCORES, BS], bf)
                ccv = cc_out.rearrange("(r k p) m -> p r k m", r=NCORES,
                                       k=KT)
                for r in range(NCORES):
                    nc.sync.dma_start(hT[:, :, r, :], ccv[:, r])

                # bias row -> broadcast to 128 partitions via K=1 matmul
                bias_full = bfp.tile([128, VP], f32)
                for c0, cs in CHUNKS:
                    psb = lps.tile([128, 512], f32, tag="psb", bufs=2)
                    nc.tensor.matmul(psb[:, :cs], ones[:],
                                     by_row[:, c0:c0 + cs],
                                     start=True, stop=True)
                    nc.vector.tensor_copy(bias_full[:, c0:c0 + cs],
                                          psb[:, :cs])

                for g0, gs in GROUPS:
                    wyT = wyp.tile([128, KT, 2048], bf, tag="wyT")
                    for k in range(KT):
                        nc.scalar.dma_start(
                            wyT[:, k, :gs],
                            wy_d.ap()[g0:g0 + gs, k * 128:(k + 1) * 128],
                            transpose=True)
                    for sub in range(gs // 512 if gs >= 512 else 1):
                        s0 = sub * 512
                        cs = min(512, gs - s0)
                        c0 = g0 + s0
                        out_t = stg.tile([128, B // 128, 512], f32,
                                         tag="out")
                        for mt in range(B // 128):
                            r, ml = divmod(mt, MT)
                            ps = lps.tile([128, 512], f32, tag="ps", bufs=6)
                            for k in range(KT):
                                nc.tensor.matmul(
                                    ps[:, :cs],
                                    hT[:, k, r, ml * 128:(ml + 1) * 128],
                                    wyT[:, k, s0:s0 + cs],
                                    start=(k == 0), stop=(k == KT - 1))
                            nc.vector.tensor_tensor(
                                out_t[:, mt, :cs], ps[:, :cs],
                                bias_full[:, c0:c0 + cs],
                                mybir.AluOpType.add)
                        nc.sync.dma_start(
                            logits_d.ap()[:, c0:c0 + cs].rearrange(
                                "(mt p) c -> p mt c", p=128),
                            out_t[:, :, :cs])

    nc.compile()
    return nc


def _get_runner():
    """Build (once) a persistent jitted SPMD runner over the 8 cores."""
    if "runner" in _STATE:
        return _STATE["runner"]

    import jax
    import jax.numpy as jnp
    import concourse.mybir as mybir
    from jax.sharding import Mesh, PartitionSpec, NamedSharding
    from jax.experimental.shard_map import shard_map
    from concourse import bass2jax

    nc = _build_program()
    bass2jax.install_neuronx_cc_hook()

    partition_name = (nc.partition_id_tensor.name
                      if nc.partition_id_tensor else None)
    in_names, out_names, out_avals = [], [], []
    for alloc in nc.m.functions[0].allocations:
        if not isinstance(alloc, mybir.MemoryLocationSet):
            continue
        name = alloc.memorylocations[0].name
        if alloc.kind == "ExternalInput":
            if name != partition_name:
                in_names.append(name)
        elif alloc.kind == "ExternalOutput":
            out_names.append(name)
            out_avals.append(jax.core.ShapedArray(
                tuple(alloc.tensor_shape), mybir.dt.np(alloc.dtype)))

    n_params = len(in_names)
    n_outs = len(out_avals)
    all_in_names = list(in_names) + list(out_names)
    if partition_name is not None:
        all_in_names.append(partition_name)

    # which inputs are identical on all cores (replicated)?
    REPLICATED = {"WxiT", "WhiT", "WxcT", "WhcT", "WxoT", "WhoT",
                  "bhi", "bhc", "bho"}

    def _body(*args):
        operands = list(args)
        if partition_name is not None:
            operands.append(bass2jax.partition_id_tensor())
        outs = bass2jax._bass_exec_p.bind(
            *operands,
            out_avals=tuple(out_avals),
            in_names=tuple(all_in_names),
            out_names=tuple(out_names),
            lowering_input_output_aliases=(),
            sim_require_finite=True,
            sim_require_nnan=True,
            nc=nc,
        )
        return tuple(outs)

    devices = jax.devices()[:NCORES]
    mesh = Mesh(np.asarray(devices), ("core",))
    shard = NamedSharding(mesh, PartitionSpec("core"))
    repl = NamedSharding(mesh, PartitionSpec())

    in_specs = tuple(
        PartitionSpec() if n in REPLICATED else PartitionSpec("core")
        for n in in_names) + (PartitionSpec("core"),) * n_outs
    out_specs = (PartitionSpec("core"),) * n_outs

    # No donation: the kernel writes every output element, so the custom
    # call can allocate outputs itself and we can reuse one set of zero
    # buffers for every invocation (keeps the hot path free of 400MB
    # zero-fills).
    sharded = jax.jit(
        shard_map(_body, mesh=mesh, in_specs=in_specs, out_specs=out_specs,
                  check_rep=False),
        keep_unused=True)

    zero_fn = jax.jit(
        lambda: tuple(
            jnp.zeros((NCORES * a.shape[0], *a.shape[1:]), a.dtype)
            for a in out_avals),
        out_shardings=tuple(shard for _ in out_avals))

    def put(name, arr):
        return jax.device_put(
            np.asarray(arr), repl if name in REPLICATED else shard)

    def make_repeat(n):
        """jit that runs the NEFF n times back-to-back on-device (bass_exec
        is effect-ordered, so repeats are neither DCE'd nor reordered)."""

        def _body_n(*args):
            outs = None
            for _ in range(n):
                outs = _body(*args)
            return outs

        return jax.jit(
            shard_map(_body_n, mesh=mesh, in_specs=in_specs,
                      out_specs=out_specs, check_rep=False),
            keep_unused=True)

    runner = {
        "in_names": in_names,
        "out_names": out_names,
        "sharded": sharded,
        "zero_fn": zero_fn,
        "put": put,
        "make_repeat": make_repeat,
    }
    _STATE["runner"] = runner
    return runner


def _t8(a):
    """[B, H] -> per-core transposed shards stacked: [8*H, BS] bf16."""
    at = a.astype(BF16).T  # [H, B]
    return np.ascontiguousarray(
        at.reshape(H, NCORES, BS).transpose(1, 0, 2)).reshape(NCORES * H, BS)


def _prep_inputs(x, h, c, emb, Wxi, Whi, bhi, Wxc, Whc, bhc, Wxo, Who, bho,
                 Wy, by):
    """Host-side shard/layout prep. Returns {name: global array}."""
    x = np.asarray(x)
    emb = np.asarray(emb, dtype=np.float32)
    xe = emb[x]  # [B, E] gather
    h = np.asarray(h, dtype=np.float32)
    c = np.asarray(c, dtype=np.float32)

    wy_pad = np.zeros((VPAD, H), dtype=BF16)
    wy_pad[:V] = np.asarray(Wy, dtype=np.float32).astype(BF16)
    by_pad = np.zeros((NCORES, VP), dtype=BF16)
    by_pad.reshape(-1)[:V] = np.asarray(by, dtype=np.float32).astype(BF16)

    def wT(a):
        return np.ascontiguousarray(
            np.asarray(a, np.float32).astype(BF16).T)

    feed = {
        "xeT_b": _t8(xe),
        "hT_b": _t8(h),
        "cT_b": _t8(c),
        "c_f": c,
        "Wy_b": wy_pad,
        "by_b": by_pad,
        "WxiT": wT(Wxi), "WhiT": wT(Whi), "WxcT": wT(Wxc),
        "WhcT": wT(Whc), "WxoT": wT(Wxo), "WhoT": wT(Who),
        "bhi": np.asarray(bhi, np.float32).astype(BF16).reshape(1, H),
        "bhc": np.asarray(bhc, np.float32).astype(BF16).reshape(1, H),
        "bho": np.asarray(bho, np.float32).astype(BF16).reshape(1, H),
    }
    return feed


def _run_device(feed_dev):
    r = _get_runner()
    if "zeros" not in _STATE:
        import jax
        _STATE["zeros"] = jax.block_until_ready(r["zero_fn"]())
    args = [feed_dev[n] for n in r["in_names"]] + list(_STATE["zeros"])
    outs = r["sharded"](*args)
    return outs


def kernel(x, h, c, emb, Wxi, Whi, bhi, Wxc, Whc, bhc, Wxo, Who, bho, Wy, by):
    r = _get_runner()
    feed = _prep_inputs(x, h, c, emb, Wxi, Whi, bhi, Wxc, Whc, bhc,
                        Wxo, Who, bho, Wy, by)
    feed_dev = {n: r["put"](n, feed[n]) for n in r["in_names"]}
    outs = _run_device(feed_dev)
    by_name = dict(zip(r["out_names"], outs))

    logits_g = np.asarray(by_name["logits"])  # [8*2048, 6400]
    logits = np.concatenate(
        [logits_g[i * B:(i + 1) * B] for i in range(NCORES)],
        axis=1)[:, :V]
    h_new = np.asarray(by_name["h_new"]).reshape(B, H)
    c_new = np.asarray(by_name["c_new"]).reshape(B, H)
    return logits.astype(np.float32), h_new, c_new
